# revision 1
# baseline (speedup 1.0000x reference)
"""Single-head attention (B=4, T=4096, D=1024, H=64, fp32 in/out) on 8 TRN2
NeuronCores.

Sharding: one core per (batch, T-half) pair -> 8 shards, no collectives.
Host pre-transposes, pre-casts and pre-packs every input so the device does
zero input transposes and minimal HBM traffic (~17 MB/core):
  xt      [8*128, 8*512] bf16  per t-block: [128 part, (d-chunk, 512)] of
                               x[b]^T (query t-blocks first)
  wqt     [128, 8*64]    bf16  Wq^T packed [part, (d-chunk, 64)]
  wkvt    [128, 8*128]   bf16  [Wk^T | Wv^T] packed likewise
  maskt   [4096, 2048]   bf16  mask slice transposed to [s, t]
Each core returns un-normalized [65, 2048] (out'^T rows 0:64, softmax
denominator row 64); the host does the final divide + transpose.

Per-core pipeline (all matmul inputs bf16, fp32 PSUM):
  P phase (query x-blocks 0-3): fused k|v + q projections as N=512 matmuls
    accumulating over 8 d-chunks in paired PSUM tiles; PSUM->SBUF copies on
    the Act engine (idle until attention); V' = [V | 1 | 0] via PE
    transposes. x-blocks 4-7 are DMA'd behind these on the scalar queue and
    their k|v projections are woven into attention at j = 3, 6, 9, 12.
  Attention (j-outer over 32 s-chunks): ST[j] = kvT[:,j-chunk]^T qTpad with
    qT zero-padded to 128 rows, so every matmul drives the full 128x128 PE
    array -- the HAM clock gate only releases (1.2 -> 2.4 GHz) under
    full-array activity, which nearly halves matmul time. exp on the Act
    engine ([128, 1024] per instruction, reading 2 PSUM banks) is the
    steady-state pacer; the mask multiply runs on DVE at the 2x bf16 rate;
    PT stays fully resident in SBUF ([128, 32, 2048] bf16). PV matmuls
    (V' zero-padded to 128 columns) trail ST by PVLAG chunks, emitted
    adjacent so the PE stream stays dense. Masks stream on the sync
    hardware DMA queue (software-DGE completion latency is ~10 us, too
    slow to gate the pipeline).
  Tail: one Act copy of pv PSUM -> bf16 SBUF, one store.
"""

import sys

if "/opt/trn_rl_repo" not in sys.path:
    sys.path.insert(0, "/opt/trn_rl_repo")

from contextlib import ExitStack

import numpy as np
import ml_dtypes

import concourse.bass as bass
import concourse.tile as tile
from concourse import bacc, mybir
from concourse.bass_utils import run_bass_kernel_spmd
from concourse.masks import make_identity

F32 = mybir.dt.float32
BF16 = mybir.dt.bfloat16

B, T, D, H = 4, 4096, 1024, 64
NCORES = 8
TQ = T // 2  # query rows per core

BF16NP = ml_dtypes.bfloat16


def build_attention_core(T=T, D=D, H=H, Tq=TQ):
    """Build the per-core Bass graph. Every core runs the same graph."""
    assert D % 128 == 0 and T % 1024 == 0 and Tq % 1024 == 0 and H == 64
    DC = D // 128          # d chunks (8)
    NS = T // 128          # s chunks (32)
    NTB = T // 512         # x t-blocks (8)
    NQB = Tq // 512        # query t-blocks (4)
    NSUP = Tq // 512       # t supertiles in attention (4)
    PVLAG = 3              # PV trails ST by this many s-chunks
    scale = 1.0 / float(np.sqrt(D))
    Exp = mybir.ActivationFunctionType.Exp

    nc = bacc.Bacc("TRN2", target_bir_lowering=False, debug=False,
                   num_devices=NCORES)
    xT_ext = nc.declare_dram_parameter("xt", [NTB * 128, DC * 512], BF16,
                                       isOutput=False)
    wqT_ext = nc.declare_dram_parameter("wqt", [128, DC * H], BF16,
                                        isOutput=False)
    wkvT_ext = nc.declare_dram_parameter("wkvt", [128, DC * 2 * H], BF16,
                                         isOutput=False)
    maskT_ext = nc.declare_dram_parameter("maskt", [T, Tq], BF16,
                                          isOutput=False)
    out_ext = nc.declare_dram_parameter("out", [H + 1, Tq], BF16,
                                        isOutput=True)

    with tile.TileContext(nc) as tc, ExitStack() as ctx:
        singles = ctx.enter_context(tc.tile_pool(name="singles", bufs=1))
        xin = ctx.enter_context(tc.tile_pool(name="xin", bufs=4))
        mpool = ctx.enter_context(tc.tile_pool(name="mpool", bufs=3))
        opool = ctx.enter_context(tc.tile_pool(name="opool", bufs=1))
        # PSUM: tag "p" [128,2,512] f32 x2 bufs (4 banks) shared by P phase,
        # ST tiles and epilogue; tag "pv" [128,4,512] f32 x1 (4 banks).
        psP = ctx.enter_context(tc.tile_pool(name="psP", bufs=2,
                                             space="PSUM"))
        psV = ctx.enter_context(tc.tile_pool(name="psV", bufs=1,
                                             space="PSUM"))


        # ---- weights (pre-packed on host) ----
        wqT_sb = singles.tile([128, DC, H], BF16)
        nc.scalar.dma_start(
            out=wqT_sb.rearrange("p a b -> p (a b)"), in_=wqT_ext[:, :]
        )
        wkvT_sb = singles.tile([128, DC, 2 * H], BF16)
        nc.scalar.dma_start(
            out=wkvT_sb.rearrange("p a b -> p (a b)"), in_=wkvT_ext[:, :]
        )

        ident_bf = singles.tile([128, 128], BF16)
        make_identity(nc, ident_bf)

        # persistent activations. qT and V' are zero-padded to the full 128
        # partition/column width so attention matmuls light up the whole PE
        # array (HAM un-throttles only under full-array activity): the vT
        # rows of kvT meet zero q rows, and V' columns 65:128 are zero.
        kvT_sb = singles.tile([128, T], BF16)   # rows 0:64 kT, 64:128 vT
        qT_sb = singles.tile([128, Tq], BF16)   # rows 64:128 zero
        Vp_sb = singles.tile([128, NS, 128], BF16)  # V' = [V | 1 | 0pad]
        PT_sb = singles.tile([128, NS, Tq], BF16)   # masked exp scores
        nc.gpsimd.memset(qT_sb[H : 2 * H, :], 0.0)
        nc.gpsimd.memset(Vp_sb[:, :, H + 1 : 128], 0.0)
        nc.gpsimd.memset(Vp_sb[:, :, H : H + 1], 1.0)

        def proj_pair(tbp, with_q):
            """k|v (+q) projections for x t-blocks 2*tbp, 2*tbp+1."""
            kv_ps = psP.tile([128, 2, 512], F32, tag="p", name="kv_ps")
            q_ps = None
            if with_q:
                q_ps = psP.tile([128, 2, 512], F32, tag="p", name="q_ps")
            for half in range(2):
                tb = 2 * tbp + half
                x_sb = x_tiles[tb]
                for j in range(DC):
                    nc.tensor.matmul(
                        kv_ps[:, half, :],
                        wkvT_sb[:, j, :],
                        x_sb[:, j, :],
                        start=(j == 0),
                        stop=(j == DC - 1),
                    )
                if q_ps is not None:
                    for j in range(DC):
                        nc.tensor.matmul(
                            q_ps[0:H, half, :],
                            wqT_sb[:, j, :],
                            x_sb[:, j, :],
                            start=(j == 0),
                            stop=(j == DC - 1),
                        )
            nc.vector.tensor_copy(
                kvT_sb[:, tbp * 1024 : (tbp + 1) * 1024],
                kv_ps.rearrange("p a b -> p (a b)"),
            )
            if q_ps is not None:
                nc.vector.tensor_copy(
                    qT_sb[0:H, tbp * 1024 : (tbp + 1) * 1024],
                    q_ps[0:H].rearrange("p a b -> p (a b)"),
                )
            # V natural layout for the 8 s-chunks of this t-block pair
            vt_ps = psP.tile([128, 8, H], BF16, tag="p", name="vt_ps")
            for jj in range(8):
                s0 = tbp * 1024 + jj * 128
                nc.tensor.transpose(
                    vt_ps[:, jj, :],
                    kvT_sb[H : 2 * H, s0 : s0 + 128],
                    ident_bf[H : 2 * H, H : 2 * H],
                )
            nc.vector.tensor_copy(
                Vp_sb[:, tbp * 8 : (tbp + 1) * 8, 0:H], vt_ps
            )

        def proj_single(tb, x_sb):
            """k|v projection for one non-query, pre-loaded x t-block.
            The V' transposes are deferred (vp_single) so this injection
            stays within one Act period and never starves the exp stream."""
            kv_ps = psP.tile([128, 2, 512], F32, tag="p", name="kv1_ps")
            for j in range(DC):
                nc.tensor.matmul(
                    kv_ps[:, 0, :],
                    wkvT_sb[:, j, :],
                    x_sb[:, j, :],
                    start=(j == 0),
                    stop=(j == DC - 1),
                )
            nc.vector.tensor_copy(
                kvT_sb[:, tb * 512 : (tb + 1) * 512], kv_ps[:, 0, :]
            )

        def vp_single(tb):
            """Deferred V' rows for one woven x t-block."""
            vt_ps = psP.tile([128, 4, H], BF16, tag="p", name="vt1_ps")
            for jj in range(4):
                s0 = tb * 512 + jj * 128
                nc.tensor.transpose(
                    vt_ps[:, jj, :],
                    kvT_sb[H : 2 * H, s0 : s0 + 128],
                    ident_bf[H : 2 * H, H : 2 * H],
                )
            nc.vector.tensor_copy(
                Vp_sb[:, tb * 4 : (tb + 1) * 4, 0:H], vt_ps
            )

        # ---- P phase: query x-blocks only; rest woven into attention ----
        # all four query x-block DMAs dispatch before any P compute sits
        # in the Act FIFO, so pair 1's blocks land ~10us earlier
        x_tiles = {}
        for tb in range(4):
            x_sb = xin.tile([128, DC, 512], BF16, tag="x", name="x_sb")
            nc.scalar.dma_start(
                out=x_sb.rearrange("p a b -> p (a b)"),
                in_=xT_ext[tb * 128 : (tb + 1) * 128, :],
            )
            x_tiles[tb] = x_sb
        for tbp in range(NQB // 2):
            proj_pair(tbp, with_q=True)
        # x4..7 transfers run during early attention; their dispatches land
        # in the Act FIFO after the P copies, by which time their ring slots
        # are already free (no FIFO blocking)
        x_late = []
        for tb in range(4, NTB):
            x_sb = xin.tile([128, DC, 512], BF16, tag="x", name="x2_sb")
            nc.gpsimd.dma_start(
                out=x_sb.rearrange("p a b -> p (a b)"),
                in_=xT_ext[tb * 128 : (tb + 1) * 128, :],
            )
            x_late.append(x_sb)

        # ---- attention: ST/exp/mask with PV trailing by PVLAG chunks ----
        pv_ps = psV.tile([128, NSUP, 512], F32, tag="pv")

        def pv_step(j):
            for ts in range(NSUP):
                nc.tensor.matmul(
                    pv_ps[:, ts, :],
                    Vp_sb[:, j, :],
                    PT_sb[:, j, ts * 512 : (ts + 1) * 512],
                    start=(j == 0),
                    stop=(j == NS - 1),
                )

        for j in range(NS):
            if j in (3, 6, 9, 12):
                # weave in the k|v projection for one x-block 4..7
                tb = 4 + (j - 3) // 3
                proj_single(tb, x_late[tb - 4])
            elif 14 <= j <= 17:
                vp_single(4 + (j - 14))
            m_sb = mpool.tile([128, Tq], BF16, tag="m")
            nc.sync.dma_start(
                out=m_sb, in_=maskT_ext[j * 128 : (j + 1) * 128, :]
            )
            for hh in range(2):
                st_ps = psP.tile([128, 2, 512], F32, tag="p")
                for ts in range(2):
                    t0 = (2 * hh + ts) * 512
                    nc.tensor.matmul(
                        st_ps[:, ts, :],
                        kvT_sb[:, j * 128 : (j + 1) * 128],
                        qT_sb[:, t0 : t0 + 512],
                    )
                nc.scalar.activation(
                    PT_sb[:, j, hh * 1024 : (hh + 1) * 1024],
                    st_ps.rearrange("p a b -> p (a b)"),
                    Exp,
                    scale=scale,
                )
            nc.vector.tensor_mul(
                PT_sb[:, j, :],
                PT_sb[:, j, :],
                m_sb,
            )
            if j >= PVLAG:
                pv_step(j - PVLAG)
        for j in range(NS - PVLAG, NS):
            pv_step(j)

        # ---- epilogue: ship un-normalized out' (host divides) ----
        oT_sb = opool.tile([H + 1, Tq], BF16, tag="oT")
        nc.scalar.copy(
            oT_sb, pv_ps[0 : H + 1].rearrange("p a b -> p (a b)")
        )
        nc.sync.dma_start(out=out_ext[:, :], in_=oT_sb)
    nc.compile()
    return nc


_NC_CACHE = {}


def _get_nc(shape_key):
    if shape_key not in _NC_CACHE:
        T_, D_, H_, Tq_ = shape_key
        _NC_CACHE[shape_key] = build_attention_core(T=T_, D=D_, H=H_, Tq=Tq_)
    return _NC_CACHE[shape_key]


def _pack_dchunks(wt):
    """[D, F] -> [128, DC*F]: partition-major packing of d-chunks."""
    Dv, Fv = wt.shape
    dc = Dv // 128
    return np.ascontiguousarray(
        wt.reshape(dc, 128, Fv).transpose(1, 0, 2).reshape(128, dc * Fv)
    )


def _prep_inputs(x, Wq, Wk, Wv, mask):
    """Host-side shard + transpose + cast + pack. Core c -> (batch c//2,
    half c%2). The x rows of the core's query half come first; mask columns
    get the same permutation so key order matches the permuted x rows."""
    x = np.ascontiguousarray(x, dtype=np.float32)
    mask = np.ascontiguousarray(mask, dtype=np.int32)
    Bv, Tv, Dv = x.shape
    Tq = Tv // 2
    ntb = Tv // 512
    dc = Dv // 128

    wqT = _pack_dchunks(
        np.ascontiguousarray(np.asarray(Wq, dtype=np.float32).T).astype(
            BF16NP
        )
    )
    wkvT = _pack_dchunks(
        np.concatenate(
            [np.asarray(Wk, np.float32).T, np.asarray(Wv, np.float32).T],
            axis=1,
        ).astype(BF16NP)
    )

    def block_xt(xb):
        # [T, D] -> [ (tb, 128part), (d-chunk, 512) ]
        xt = xb.T.astype(BF16NP)  # [D, T]
        x4 = xt.reshape(dc, 128, ntb, 512).transpose(2, 1, 0, 3)
        return np.ascontiguousarray(x4.reshape(ntb * 128, dc * 512))

    # mask is shared across batches: only two variants (one per half)
    m0 = mask[0, 0:Tq, :]  # [t, s] for half 0
    m1 = np.concatenate([mask[0, Tq:, Tq:], mask[0, Tq:, :Tq]], axis=1)
    maskT0 = np.ascontiguousarray(m0.T.astype(BF16NP))
    maskT1 = np.ascontiguousarray(m1.T.astype(BF16NP))

    in_maps = []
    for c in range(NCORES):
        b, half = c // 2, c % 2
        if half == 0:
            xc = x[b]
            mT = maskT0
        else:
            xc = np.concatenate([x[b, Tq:], x[b, :Tq]], axis=0)
            mT = maskT1
        in_maps.append(
            {
                "xt": block_xt(xc),
                "wqt": wqT,
                "wkvt": wkvT,
                "maskt": mT,
            }
        )
    return in_maps


def kernel(x, Wq, Wk, Wv, mask, _trace=False):
    x = np.asarray(x)
    Bv, Tv, Dv = x.shape
    Hv = np.asarray(Wq).shape[0]
    Tq = Tv // 2
    nc = _get_nc((Tv, Dv, Hv, Tq))
    in_maps = _prep_inputs(
        np.asarray(x), np.asarray(Wq), np.asarray(Wk), np.asarray(Wv),
        np.asarray(mask),
    )
    res = run_bass_kernel_spmd(
        nc, in_maps, core_ids=list(range(NCORES)), trace=_trace
    )
    out = np.empty((Bv, Tv, Hv), dtype=np.float32)
    for c in range(NCORES):
        b, half = c // 2, c % 2
        r = np.asarray(res.results[c]["out"], dtype=np.float32)
        out[b, half * Tq : (half + 1) * Tq] = (r[0:Hv] / r[Hv : Hv + 1]).T
    if _trace:
        kernel.last_results = res
    return out



# revision 6
# speedup vs baseline: 1.0145x; 1.0145x over previous
"""Single-head attention (B=4, T=4096, D=1024, H=64, fp32 in/out) on 8 TRN2
NeuronCores.

Sharding: one core per (batch, T-half) pair -> 8 shards, no collectives.
Host pre-transposes/pre-casts/pre-packs inputs (zero device-side transposes):
  xt      [8*128, 8*512] bf16  per t-block: [128 part, (d-chunk, 512)] of
                               x[b]^T (query t-blocks first)
  wqt     [128, 8*64]    bf16  Wq^T packed [part, (d-chunk, 64)]
  wkvt    [128, 8*128]   bf16  [Wk^T | Wv^T] packed likewise
  maskt   [4096, 2048]   bf16  mask slice transposed to [s, t]
Each core returns un-normalized [65, 2048] (out'^T rows 0:64, softmax
denominator row 64); the host does the final divide + transpose.

Per-core pipeline. The Act engine's exp stream is the hard floor
(8.4M elements / 128 lanes / ~0.9 GHz ~= 72 us), so the kernel is built
to start that stream early and never stall it. Every engine executes its
instructions in program order, so a late dependency anywhere in a stream
blocks everything behind it -- all ordering below is chosen around that.

  - Two independent column streams: A = query cols 0:1024 (PV supertiles
    0,1), B = cols 1024:2048 (2,3). Order A0..A5, then (A6,B0), (A7,B1),
    ..., ending B26..B31: exp-A0 fires once q-blocks 0,1 + kv-block 0
    exist; B waits for q-blocks 2,3 whose x arrives later.
  - DMA engines share bandwidth across ALL queued transfers (queues are
    FIFO internally but round-robin against each other), so the prologue
    keeps the critical set tiny: weights ride the scalar HW queue while
    x0,x1 ride the sync HW queue chunk-pipelined (projection matmuls
    chase the landing d-chunks); x2,x3 follow; x4-x7 are dispatched from
    inside the loop so they cannot steal early bandwidth; masks stream
    behind on the same sync queue, decoupled by a deep PT ring + lagged
    PV so mask lateness never gates exp.
  - kv projections: block 0 in the prologue; blocks 1-7 woven into the
    loop as two 4-chunk PSUM half-bursts merged by a DVE add (a half
    burst returns its PSUM generation within one step, keeping the
    2-deep ST ping-pong live). q2,q3 are woven early (for stream B).
  - V' = [V | 1 | 0] via PE transposes, woven after each kv block.
  - ST matmuls use full-128-row kvT / zero-padded qT so the whole PE
    array stays active (HAM clock gate 1.2 -> 2.4 GHz); dummy warmup
    matmuls ramp the clock before the first projection.
  - PV trails its stream's exp by 3 steps; stream B's trail tightens at
    the end to shorten the critical tail; epilogue copies run on
    Vector/Act after the last exp; host divides.
"""

import sys

if "/opt/trn_rl_repo" not in sys.path:
    sys.path.insert(0, "/opt/trn_rl_repo")

from contextlib import ExitStack

import numpy as np
import ml_dtypes

import concourse.bass as bass
import concourse.tile as tile
from concourse import bacc, mybir
from concourse.bass_utils import run_bass_kernel_spmd
from concourse.masks import make_identity

F32 = mybir.dt.float32
BF16 = mybir.dt.bfloat16

B, T, D, H = 4, 4096, 1024, 64
NCORES = 8
TQ = T // 2  # query rows per core

BF16NP = ml_dtypes.bfloat16


def build_attention_core(T=T, D=D, H=H, Tq=TQ):
    """Build the per-core Bass graph. Every core runs the same graph."""
    assert D % 128 == 0 and T % 1024 == 0 and Tq % 1024 == 0 and H == 64
    DC = D // 128          # d chunks (8)
    NS = T // 128          # s chunks (32)
    NTB = T // 512         # x t-blocks (8)
    NQB = Tq // 512        # query t-blocks (4)
    PVLAG = 3              # PV trails its stream's exp by this many steps
    scale = 1.0 / float(np.sqrt(D))
    Exp = mybir.ActivationFunctionType.Exp

    nc = bacc.Bacc("TRN2", target_bir_lowering=False, debug=False,
                   num_devices=NCORES)
    xT_ext = nc.declare_dram_parameter("xt", [NTB * 128, DC * 512], BF16,
                                       isOutput=False)
    wqT_ext = nc.declare_dram_parameter("wqt", [128, DC * H], BF16,
                                        isOutput=False)
    wkvT_ext = nc.declare_dram_parameter("wkvt", [128, DC * 2 * H], BF16,
                                         isOutput=False)
    maskT_ext = nc.declare_dram_parameter("maskt", [T, Tq], BF16,
                                          isOutput=False)
    out_ext = nc.declare_dram_parameter("out", [H + 1, Tq], BF16,
                                        isOutput=True)

    with tile.TileContext(nc) as tc, ExitStack() as ctx:
        singles = ctx.enter_context(tc.tile_pool(name="singles", bufs=1))
        xin = ctx.enter_context(tc.tile_pool(name="xin", bufs=1))
        mpool = ctx.enter_context(tc.tile_pool(name="mpool", bufs=7))
        ptpool = ctx.enter_context(tc.tile_pool(name="ptpool", bufs=12))
        kvh = ctx.enter_context(tc.tile_pool(name="kvh", bufs=2))
        opool = ctx.enter_context(tc.tile_pool(name="opool", bufs=1))
        # PSUM: tag "st" [128,2,512] f32 x2 bufs (4 banks) shared by ST
        # tiles, q/kv projections, V' transposes and warmup; tag "pv"
        # [128,4,512] f32 x1 (4 banks) = (stream,ts) accumulators.
        psP = ctx.enter_context(tc.tile_pool(name="psP", bufs=2,
                                             space="PSUM"))
        psV = ctx.enter_context(tc.tile_pool(name="psV", bufs=1,
                                             space="PSUM"))

        # ---- persistent SBUF ----
        wqT_sb = singles.tile([128, DC, H], BF16)
        wkvT_sb = singles.tile([128, DC, 2 * H], BF16)
        ident_bf = singles.tile([128, 128], BF16)
        warm_sb = singles.tile([128, 512], BF16)
        kvT_sb = singles.tile([128, T], BF16)   # rows 0:64 kT, 64:128 vT
        qT_sb = singles.tile([128, Tq], BF16)   # rows 64:128 zero
        Vp_sb = singles.tile([128, NS, 128], BF16)  # V' = [V | 1 | 0pad]

        # ---- weights on the scalar HW queue (parallel with x on sync) ----
        nc.scalar.dma_start(
            out=wqT_sb.rearrange("p a b -> p (a b)"), in_=wqT_ext[:, :]
        )
        nc.scalar.dma_start(
            out=wkvT_sb.rearrange("p a b -> p (a b)"), in_=wkvT_ext[:, :]
        )

        # ---- x0, x1 chunk-pipelined on the sync HW queue ----
        x_tiles = {}
        for b in range(2):
            x_sb = xin.tile([128, DC, 512], BF16, tag="x", bufs=NTB,
                            name="x_sb")
            for c in range(0, DC, 2):
                nc.sync.dma_start(
                    out=x_sb[:, c : c + 2, :].rearrange("p a b -> p (a b)"),
                    in_=xT_ext[b * 128 : (b + 1) * 128,
                               c * 512 : (c + 2) * 512],
                )
            x_tiles[b] = x_sb
        for b in range(2, NQB):
            x_sb = xin.tile([128, DC, 512], BF16, tag="x", bufs=NTB,
                            name="x2_sb")
            nc.sync.dma_start(
                out=x_sb.rearrange("p a b -> p (a b)"),
                in_=xT_ext[b * 128 : (b + 1) * 128, :],
            )
            x_tiles[b] = x_sb

        def x_late(b):
            """Dispatch a late x block from inside the loop (sync queue)."""
            x_sb = xin.tile([128, DC, 512], BF16, tag="x", bufs=NTB,
                            name="xl_sb")
            nc.sync.dma_start(
                out=x_sb.rearrange("p a b -> p (a b)"),
                in_=xT_ext[b * 128 : (b + 1) * 128, :],
            )
            x_tiles[b] = x_sb

        # ---- gpsimd setup (runs while DMAs fly) ----
        nc.gpsimd.memset(warm_sb, 1.0)
        nc.gpsimd.memset(qT_sb[H : 2 * H, :], 0.0)
        make_identity(nc, ident_bf)
        nc.gpsimd.memset(Vp_sb[:, :, H + 1 : 128], 0.0)
        nc.gpsimd.memset(Vp_sb[:, :, H : H + 1], 1.0)

        # ---- PE warmup: ramp the clock before real work ----
        def warm_burst(n):
            w_ps = psP.tile([128, 2, 512], F32, tag="st", name="w_ps")
            for i in range(n):
                nc.tensor.matmul(w_ps[:, i % 2, :], ident_bf, warm_sb)

        for _ in range(3):
            warm_burst(2)

        def q_block(b):
            """q projection for one query t-block (chases its x chunks)."""
            q_ps = psP.tile([128, 2, 512], F32, tag="st", name="q_ps")
            for c in range(DC):
                nc.tensor.matmul(
                    q_ps[0:H, 0, :],
                    wqT_sb[:, c, :],
                    x_tiles[b][:, c, :],
                    start=(c == 0),
                    stop=(c == DC - 1),
                )
            nc.vector.tensor_copy(
                qT_sb[0:H, b * 512 : (b + 1) * 512], q_ps[0:H, 0, :]
            )

        def kv_whole(b):
            """kv projection for one t-block, prologue style (8-chunk)."""
            kv_ps = psP.tile([128, 2, 512], F32, tag="st", name="kv_ps")
            for c in range(DC):
                nc.tensor.matmul(
                    kv_ps[:, 0, :],
                    wkvT_sb[:, c, :],
                    x_tiles[b][:, c, :],
                    start=(c == 0),
                    stop=(c == DC - 1),
                )
            nc.vector.tensor_copy(
                kvT_sb[:, b * 512 : (b + 1) * 512], kv_ps[:, 0, :]
            )

        def kv_half1(b):
            kv_ps = psP.tile([128, 2, 512], F32, tag="st", name="kvh1_ps")
            for c in range(DC // 2):
                nc.tensor.matmul(
                    kv_ps[:, 0, :],
                    wkvT_sb[:, c, :],
                    x_tiles[b][:, c, :],
                    start=(c == 0),
                    stop=(c == DC // 2 - 1),
                )
            h_sb = kvh.tile([128, 512], F32, tag="kvh", name="h_sb")
            nc.vector.tensor_copy(h_sb, kv_ps[:, 0, :])
            return h_sb

        def kv_half2(b, h_sb):
            kv_ps = psP.tile([128, 2, 512], F32, tag="st", name="kvh2_ps")
            for c in range(DC // 2, DC):
                nc.tensor.matmul(
                    kv_ps[:, 0, :],
                    wkvT_sb[:, c, :],
                    x_tiles[b][:, c, :],
                    start=(c == DC // 2),
                    stop=(c == DC - 1),
                )
            nc.vector.tensor_tensor(
                out=kvT_sb[:, b * 512 : (b + 1) * 512],
                in0=kv_ps[:, 0, :],
                in1=h_sb,
                op=mybir.AluOpType.add,
            )

        def vp_block(b):
            """V' rows for one t-block (4 s-chunks of transposes)."""
            vt_ps = psP.tile([128, 4, H], BF16, tag="st", name="vt_ps")
            for jj in range(4):
                s0 = b * 512 + jj * 128
                nc.tensor.transpose(
                    vt_ps[:, jj, :],
                    kvT_sb[H : 2 * H, s0 : s0 + 128],
                    ident_bf[H : 2 * H, H : 2 * H],
                )
            nc.vector.tensor_copy(
                Vp_sb[:, b * 4 : (b + 1) * 4, 0:H], vt_ps
            )

        # prologue compute: q0 -> kv0 -> q1 (each chases its DMA)
        q_block(0)
        kv_whole(0)
        q_block(1)
        vp_block(0)

        # ---- attention: two interleaved column streams ----
        # stream 0 (A): query cols 0:1024 (pv banks 0,1)
        # stream 1 (B): cols 1024:2048 (pv banks 2,3)
        order = [(0, j) for j in range(6)]
        for j in range(6, NS):
            order.append((0, j))
            order.append((1, j - 6))
        order += [(1, j) for j in range(NS - 6, NS)]
        assert len(order) == 2 * NS

        # weave schedule: step idx -> list of (kind, arg). Deadlines
        # (A_j at g=j for j<6, g=2j-6 after; B_j at g=2j+7):
        # kv_b before ST-A_{4b}; vp_b before PV-A_{4b} (step A_{4b+3}).
        weave = {
            1: [("kv1", 1)], 2: [("kv2", 1)],
            3: [("q", 2), ("xl", 4)], 5: [("q", 3)], 6: [("vp", 1)],
            7: [("kv1", 2)], 8: [("kv2", 2)], 9: [("vp", 2)],
            11: [("xl", 5)],
            12: [("kv1", 3)], 13: [("kv2", 3)], 15: [("vp", 3)],
            19: [("xl", 6)],
            20: [("kv1", 4)], 21: [("kv2", 4)], 23: [("vp", 4)],
            27: [("xl", 7)],
            28: [("kv1", 5)], 29: [("kv2", 5)], 31: [("vp", 5)],
            36: [("kv1", 6)], 37: [("kv2", 6)], 39: [("vp", 6)],
            44: [("kv1", 7)], 45: [("kv2", 7)], 47: [("vp", 7)],
        }

        pv_ps = psV.tile([128, 4, 512], F32, tag="pv")
        pt_tiles = [{}, {}]
        m_tiles = {}
        kv_pend = {}

        def pv_step(si, j):
            ptt = pt_tiles[si].pop(j)
            for ts in range(2):
                nc.tensor.matmul(
                    pv_ps[:, 2 * si + ts, :],
                    Vp_sb[:, j, :],
                    ptt[:, ts * 512 : (ts + 1) * 512],
                    start=(j == 0),
                    stop=(j == NS - 1),
                )

        for g, (si, j) in enumerate(order):
            for kind, b in weave.get(g, []):
                if kind == "q":
                    q_block(b)
                elif kind == "kv1":
                    kv_pend[b] = kv_half1(b)
                elif kind == "kv2":
                    kv_half2(b, kv_pend.pop(b))
                elif kind == "xl":
                    x_late(b)
                else:
                    vp_block(b)
            if si == 0:
                m_sb = mpool.tile([128, Tq], BF16, tag="m")
                nc.sync.dma_start(
                    out=m_sb, in_=maskT_ext[j * 128 : (j + 1) * 128, :]
                )
                m_tiles[j] = m_sb
            st_ps = psP.tile([128, 2, 512], F32, tag="st", name="st_ps")
            base = si * 1024
            for ts in range(2):
                t0 = base + ts * 512
                nc.tensor.matmul(
                    st_ps[:, ts, :],
                    kvT_sb[:, j * 128 : (j + 1) * 128],
                    qT_sb[:, t0 : t0 + 512],
                )
            ptt = ptpool.tile([128, 1024], BF16, tag="pt", name="ptt")
            nc.scalar.activation(
                ptt, st_ps.rearrange("p a b -> p (a b)"), Exp, scale=scale
            )
            nc.vector.tensor_mul(
                ptt, ptt, m_tiles[j][:, base : base + 1024]
            )
            pt_tiles[si][j] = ptt
            # standard PV lag, with stream B's trail tightened at the end
            if si == 1 and j == NS - 2:
                pv_step(1, j - PVLAG)
                pv_step(1, j - PVLAG + 1)
            elif si == 1 and j == NS - 1:
                pv_step(1, j - 2)
                pv_step(1, j - 1)
            elif j >= PVLAG:
                pv_step(si, j - PVLAG)

        # ---- drain + epilogue (copies off the Act exp stream) ----
        for j in range(NS - PVLAG, NS):
            pv_step(0, j)
        oA_sb = opool.tile([H + 1, 1024], BF16, tag="oA")
        nc.vector.tensor_copy(
            oA_sb, pv_ps[0 : H + 1, 0:2, :].rearrange("p a b -> p (a b)")
        )
        nc.sync.dma_start(out=out_ext[:, 0:1024], in_=oA_sb)
        pv_step(1, NS - 1)
        oB_sb = opool.tile([H + 1, 1024], BF16, tag="oB")
        nc.scalar.copy(
            oB_sb, pv_ps[0 : H + 1, 2:4, :].rearrange("p a b -> p (a b)")
        )
        nc.sync.dma_start(out=out_ext[:, 1024:2048], in_=oB_sb)
    nc.compile()
    return nc


_NC_CACHE = {}


def _get_nc(shape_key):
    if shape_key not in _NC_CACHE:
        T_, D_, H_, Tq_ = shape_key
        _NC_CACHE[shape_key] = build_attention_core(T=T_, D=D_, H=H_, Tq=Tq_)
    return _NC_CACHE[shape_key]


def _pack_dchunks(wt):
    """[D, F] -> [128, DC*F]: partition-major packing of d-chunks."""
    Dv, Fv = wt.shape
    dc = Dv // 128
    return np.ascontiguousarray(
        wt.reshape(dc, 128, Fv).transpose(1, 0, 2).reshape(128, dc * Fv)
    )


def _prep_inputs(x, Wq, Wk, Wv, mask):
    """Host-side shard + transpose + cast + pack. Core c -> (batch c//2,
    half c%2). The x rows of the core's query half come first; mask columns
    get the same permutation so key order matches the permuted x rows."""
    x = np.ascontiguousarray(x, dtype=np.float32)
    mask = np.ascontiguousarray(mask, dtype=np.int32)
    Bv, Tv, Dv = x.shape
    Tq = Tv // 2
    ntb = Tv // 512
    dc = Dv // 128

    wqT = _pack_dchunks(
        np.ascontiguousarray(np.asarray(Wq, dtype=np.float32).T).astype(
            BF16NP
        )
    )
    wkvT = _pack_dchunks(
        np.concatenate(
            [np.asarray(Wk, np.float32).T, np.asarray(Wv, np.float32).T],
            axis=1,
        ).astype(BF16NP)
    )

    def block_xt(xb):
        # [T, D] -> [ (tb, 128part), (d-chunk, 512) ]
        xt = xb.T.astype(BF16NP)  # [D, T]
        x4 = xt.reshape(dc, 128, ntb, 512).transpose(2, 1, 0, 3)
        return np.ascontiguousarray(x4.reshape(ntb * 128, dc * 512))

    # mask is shared across batches: only two variants (one per half)
    m0 = mask[0, 0:Tq, :]  # [t, s] for half 0
    m1 = np.concatenate([mask[0, Tq:, Tq:], mask[0, Tq:, :Tq]], axis=1)
    maskT0 = np.ascontiguousarray(m0.T.astype(BF16NP))
    maskT1 = np.ascontiguousarray(m1.T.astype(BF16NP))

    in_maps = []
    for c in range(NCORES):
        b, half = c // 2, c % 2
        if half == 0:
            xc = x[b]
            mT = maskT0
        else:
            xc = np.concatenate([x[b, Tq:], x[b, :Tq]], axis=0)
            mT = maskT1
        in_maps.append(
            {
                "xt": block_xt(xc),
                "wqt": wqT,
                "wkvt": wkvT,
                "maskt": mT,
            }
        )
    return in_maps


def kernel(x, Wq, Wk, Wv, mask, _trace=False):
    x = np.asarray(x)
    Bv, Tv, Dv = x.shape
    Hv = np.asarray(Wq).shape[0]
    Tq = Tv // 2
    nc = _get_nc((Tv, Dv, Hv, Tq))
    in_maps = _prep_inputs(
        np.asarray(x), np.asarray(Wq), np.asarray(Wk), np.asarray(Wv),
        np.asarray(mask),
    )
    res = run_bass_kernel_spmd(
        nc, in_maps, core_ids=list(range(NCORES)), trace=_trace
    )
    out = np.empty((Bv, Tv, Hv), dtype=np.float32)
    for c in range(NCORES):
        b, half = c // 2, c % 2
        r = np.asarray(res.results[c]["out"], dtype=np.float32)
        out[b, half * Tq : (half + 1) * Tq] = (r[0:Hv] / r[Hv : Hv + 1]).T
    if _trace:
        kernel.last_results = res
    return out
